# revision 74
# baseline (speedup 1.0000x reference)
"""Transformer encoder layer (Informer-style) Bass/Tile kernel for TRN2.

Final version: 458.9us per core (TimelineSim), from a 571.9us baseline (458916ns).
Data-parallel over batch: one [S=1024, D=1024] layer per NeuronCore x8.

What it took (largest first):
 - bf16 staging of every matmul operand (hsT, all weights, probs, v_aug,
   attnC, hT, x1T): halves DMA bytes and SBUF footprint; PE rate is
   unchanged (bf16 moving operand = 1.0 cycles/row, same as fp32r >=256).
 - one strided dma_start per tensor instead of per tile: per-DMA issue
   costs ~1.3us of SEQ+HWDGE time, which throttled startup far below the
   360GB/s DMA bandwidth.
 - all dma_starts ride the sync (SP) queue: a dma_start parked at a queue
   head blocks that SEQ, and the scalar queue IS the Activation
   sequencer - wqk loads there were stalling exp dispatch ~4us per
   head-pair in the ACT-limited attention phase.
 - attention software pipelining: attn(tk-3) three deep behind
   scores(tk) — at one-deep the exp->attn slack (~126ns) equals the
   semaphore propagation cost (117ns), so every tk paid one sem-prop —
   and the qk projection
   of head-pair hp+2 sliced into 16 two-matmul pieces interleaved one per
   tk slot (paced to stretch through hp=6) as PE slack work.
 - fast path for the spec's actual fills (all biases zero, LN gamma=1,
   beta=0) drops the bias adds and LN affine ops; a general variant is
   compiled lazily if runtime inputs are not identity.
 - x1 kept resident in SBUF in bf16; the d-major x1T for fc1 comes from
   the DMA XBAR transpose (was: 64 PE transposes + 64 PSUM-evict copies),
   split into two half-tiles so fc1's first chains do not wait on ts=7's
   late transpose.
 - FFN in 4 f-quarters with 8-step PSUM chains; the last quarter PRELOADS
   out2+x1 into PSUM via DVE and accumulates on top (start=False), so LN2
   stats/xhat read the finished PSUM directly and the kernel tail is one
   tile's stats+xhat+store (~7us; was 36us of serialized LN2).
 - head eviction divides straight from PSUM per 512-chunk (reciprocal of
   the denominator row -> gpsimd partition_broadcast -> DVE multiply).

SBUF slot plan (per-partition KB, budget ~208):
  resident: const ~.6 | f2wp 16 (wv -> w2 quarters) | f1wp 16 |
            big16 2x16 (hsT -> attnC -> x1T_a) | big32 32 (hs -> out2) |
            x1bp 24 (x1 bf16 + x1T_b)
  phase:    wo 16 + vaug 16.25 (+fused ~37 in attention; tmpC ~13 in
            out-proj; hTp 32 + tmpE 8 in FFN)
"""

from contextlib import ExitStack

import concourse.bass as bass
import concourse.mybir as mybir
import concourse.tile as tile
from concourse import bacc

AFT = mybir.ActivationFunctionType
ALU = mybir.AluOpType
F32 = mybir.dt.float32
F32R = mybir.dt.float32r
BF16 = mybir.dt.bfloat16

P = 128
S = 1024
D = 1024
H = 16
HD = 64
F = 4096
NTS = S // P   # 8
NTD = D // P   # 8
FQ = 1024      # fc1/fc2 f-quarter size
NQ = F // FQ   # 4
FQT = FQ // P  # 8
EPS = 1e-5
NCH = 2
CW = 512


def build(fast=True):
    nc = bacc.Bacc("TRN2", target_bir_lowering=False, debug=False)

    def din(name, shape, dt=F32):
        return nc.dram_tensor(name, shape, dt, kind="ExternalInput").ap()

    io = dict(
        hsT=din("hsT", (D, S), BF16),
        hs=din("hs", (S, D)),
        # wq/wk in hp-blocked, per-partition-contiguous layout:
        # wqb[hp][p, ti*P + c] = (wq.T * SCALING)[ti*P + p, hp*P + c].
        # 2048-byte descriptor runs instead of 256-byte (which pay a 2x
        # small-element DMA penalty).
        wqb=din("wqb", (H // 2, P, D), BF16),
        wkb=din("wkb", (H // 2, P, D), BF16),
        wvT=din("wvT", (D, D), BF16),
        woT=din("woT", (D, D), BF16),
        bq=din("bq", (D,)),             # * SCALING
        bk=din("bk", (D,)),
        bv=din("bv", (D,)),
        bo=din("bo", (D,)),
        g1=din("g1", (D,)),
        b1=din("b1", (D,)),
        g2=din("g2", (D,)),
        b2=din("b2", (D,)),
        f1w=din("f1w", (D, F), BF16),   # fc1_w.T
        f1b=din("f1b", (F,)),
        f2w=din("f2w", (F, D), BF16),   # fc2_w.T
        f2b=din("f2b", (D,)),
        out=nc.dram_tensor("out", (S, D), F32, kind="ExternalOutput").ap(),
        fast=fast,
    )

    with tile.TileContext(nc) as tc:
        _body(tc, io)
    nc.compile()
    return nc


def _body(tc, t):
    nc = tc.nc
    fast = t["fast"]
    hsT, hs = t["hsT"], t["hs"]
    wqb, wkb, wvT, woT = t["wqb"], t["wkb"], t["wvT"], t["woT"]
    bq, bk, bv, bo = t["bq"], t["bk"], t["bv"], t["bo"]
    g1, b1, g2, b2 = t["g1"], t["b1"], t["g2"], t["b2"]
    f1w, f1b, f2w, f2b = t["f1w"], t["f1b"], t["f2w"], t["f2b"]
    out = t["out"]

    # ---- pools, in LIFO release order (bottom of stack first) ----
    const = tc.alloc_tile_pool(name="const", bufs=1)
    f2wp = tc.alloc_tile_pool(name="f2wp", bufs=1)
    f1wp = tc.alloc_tile_pool(name="f1wp", bufs=1)
    big16 = tc.alloc_tile_pool(name="big16", bufs=2)
    big32 = tc.alloc_tile_pool(name="big32", bufs=1)
    x1bp = tc.alloc_tile_pool(name="x1bp", bufs=1)
    psU = tc.alloc_tile_pool(name="psU", bufs=3, space="PSUM")
    psQ = tc.alloc_tile_pool(name="psQ", bufs=2, space="PSUM")
    wo_pool = tc.alloc_tile_pool(name="wo_pool", bufs=1)
    vaug_pool = tc.alloc_tile_pool(name="vaug_pool", bufs=1)

    eps_t = const.tile([P, 1], F32, tag="eps")
    nc.vector.memset(eps_t, EPS)
    ones_t = const.tile([P, 1], F32, tag="ones")
    nc.vector.memset(ones_t, 1.0)
    if not fast:
        bqk_t = const.tile([P, 2, NTD], F32, tag="bqk")
        nc.sync.dma_start(out=bqk_t[:, 0, :], in_=bq.rearrange("(t p) -> p t", p=P))
        nc.sync.dma_start(out=bqk_t[:, 1, :], in_=bk.rearrange("(t p) -> p t", p=P))
        f1b_t = const.tile([P, F // P], F32, tag="f1b")
        nc.sync.dma_start(out=f1b_t, in_=f1b.rearrange("(t p) -> p t", p=P))

    def bcast_tile(pool, src, queue=None):
        bt = pool.tile([P, D], F32, tag="bc" + src.name, name="bc" + src.name)
        (queue or nc.sync).dma_start(
            out=bt, in_=src.unsqueeze(0).broadcast_to((P, D)))
        return bt

    # ---- persistent tiles ----
    hsT_sb = big16.tile([P, NTD, S], BF16, tag="big", name="hsT_sb")
    hs_sb = big32.tile([P, NTS, D], F32, tag="big", name="hs_sb")
    v_aug = vaug_pool.tile([P, NTS, H, HD + 1], BF16, tag="vaug")
    wo_sb = wo_pool.tile([P, NTD, D], BF16, tag="wo")
    wv_sb = f2wp.tile([P, NTD, D], BF16, tag="w2", name="wv_sb")

    # ---- initial loads (batched: one strided dma_start per tensor — the
    # ~1.3us per-DMA issue overhead on the SEQ/HWDGE otherwise limits the
    # startup feed rate far below DMA bandwidth) ----
    hsT_r = hsT.rearrange("(t p) s -> p t s", p=P)
    wvT_r = wvT.rearrange("(t p) d -> p t d", p=P)
    hf = NTD // 2
    if not fast:
        bv_bc = bcast_tile(vaug_pool, bv, queue=nc.sync)

    f1w_r = f1w.rearrange("(t p) f -> p t f", p=P)
    f2w_r = f2w.rearrange("(t p) d -> p t d", p=P)

    def load_quarter(q):
        w1 = f1wp.tile([P, NTD, FQ], BF16, tag="w1", name=f"w1q{q}")
        nc.sync.dma_start(out=w1, in_=f1w_r[:, :, q * FQ:(q + 1) * FQ])
        w2 = f2wp.tile([P, FQT, D], BF16, tag="w2", name=f"w2q{q}")
        nc.sync.dma_start(out=w2, in_=f2w_r[:, q * FQT:(q + 1) * FQT, :])
        return w1, w2

    # ones column of v_aug (denominator trick)
    nc.vector.tensor_copy(
        out=v_aug[:, :, :, HD:HD + 1],
        in_=ones_t.unsqueeze(1).unsqueeze(1).broadcast_to((P, NTS, H, 1)))

    # ---------------- fused attention pools (alloc before qk0) ----------------
    fused = ExitStack()
    qk_pool = fused.enter_context(tc.tile_pool(name="qkt", bufs=3))
    wqk_pool = fused.enter_context(tc.tile_pool(name="wqkp", bufs=2))
    probs_pool = fused.enter_context(tc.tile_pool(name="probs", bufs=5))
    bc_pool = fused.enter_context(tc.tile_pool(name="bcp", bufs=3))
    rr_pool = fused.enter_context(tc.tile_pool(name="rrp", bufs=4))
    stg_pool = fused.enter_context(tc.tile_pool(name="stgp", bufs=2))

    def qk_issue(hp):
        st = {"hp": hp}
        for wsrc, nm in ((wqb, "q"), (wkb, "k")):
            wblk = wqk_pool.tile([P, NTD, P], BF16, tag="w" + nm, name="w" + nm)
            nc.sync.dma_start(out=wblk, in_=wsrc[hp])
            st["w" + nm] = wblk
            st[nm] = qk_pool.tile([P, S], BF16, tag=nm + "T", name=nm + "Th")
        return st

    def qk_copy(st, nm, nch, ps):
        if fast:
            nc.vector.tensor_copy(
                out=st[nm][:, nch * CW:(nch + 1) * CW], in_=ps)
        else:
            bidx = 0 if nm == "q" else 1
            nc.vector.tensor_scalar_add(
                out=st[nm][:, nch * CW:(nch + 1) * CW], in0=ps,
                scalar1=bqk_t[:, bidx, st["hp"]:st["hp"] + 1])

    def qk_compute(st, interleave=False):
        # interleave=True (head-pair 0 only): the two nch chains advance in
        # two-step pieces so each freshly-landed hsT quarter feeds ~850ns of
        # PE work instead of ~430ns — halves the DMA-feed stalls at startup.
        for nm in ("q", "k"):
            if interleave:
                pss = [psQ.tile([P, CW], F32, tag="q5", name="psq")
                       for _ in range(NCH)]
                for ti0 in range(0, NTD, 2):
                    for nch in range(NCH):
                        for ti in (ti0, ti0 + 1):
                            nc.tensor.matmul(
                                pss[nch], lhsT=st["w" + nm][:, ti, :],
                                rhs=hsT_sb[:, ti, nch * CW:(nch + 1) * CW],
                                start=(ti == 0), stop=(ti == NTD - 1))
                for nch in range(NCH):
                    qk_copy(st, nm, nch, pss[nch])
            else:
                for nch in range(NCH):
                    ps = psQ.tile([P, CW], F32, tag="q5", name="psq")
                    for ti in range(NTD):
                        nc.tensor.matmul(
                            ps, lhsT=st["w" + nm][:, ti, :],
                            rhs=hsT_sb[:, ti, nch * CW:(nch + 1) * CW],
                            start=(ti == 0), stop=(ti == NTD - 1))
                    qk_copy(st, nm, nch, ps)
        return st

    def qk_pieces(st):
        """The qk projection of head-pair hp+2 sliced into 16 two-matmul
        thunks, interleaved one per tk iteration of the current heads: the
        PE then always has slack work while ACT streams the exps, instead
        of a solid qk block during which ACT drains and then starves."""
        thunks = []
        for nm in ("q", "k"):
            for nch in range(NCH):
                holder = {}
                for ti0 in range(0, NTD, 2):
                    def piece(nm=nm, nch=nch, ti0=ti0, holder=holder, st=st):
                        if ti0 == 0:
                            holder["ps"] = psQ.tile([P, CW], F32, tag="q5",
                                                    name="psq")
                        ps = holder["ps"]
                        for ti in (ti0, ti0 + 1):
                            nc.tensor.matmul(
                                ps, lhsT=st["w" + nm][:, ti, :],
                                rhs=hsT_sb[:, ti, nch * CW:(nch + 1) * CW],
                                start=(ti == 0), stop=(ti == NTD - 1))
                        if ti0 == NTD - 2:
                            qk_copy(st, nm, nch, ps)
                    thunks.append(piece)
        return thunks

    # qk of head-pairs 0 and 1 first: they only need hsT + small weight
    # blocks. The startup loads interleave so qk0's chain can begin as soon
    # as wq0 + the first hsT quarter land and then stream; wv follows (the
    # v-proj only starts after ~14us of qk0/qk1 PE work).
    st0 = {"hp": 0}
    st0["wq"] = wqk_pool.tile([P, NTD, P], BF16, tag="wq", name="wq")
    nc.sync.dma_start(out=st0["wq"], in_=wqb[0])
    for i in range(0, 2):
        nc.sync.dma_start(out=hsT_sb[:, i, :], in_=hsT_r[:, i, :])
    st0["wk"] = wqk_pool.tile([P, NTD, P], BF16, tag="wk", name="wk")
    nc.sync.dma_start(out=st0["wk"], in_=wkb[0])
    for i in range(2, NTD):
        nc.sync.dma_start(out=hsT_sb[:, i, :], in_=hsT_r[:, i, :])
    st0["q"] = qk_pool.tile([P, S], BF16, tag="qT", name="qTh")
    st0["k"] = qk_pool.tile([P, S], BF16, tag="kT", name="kTh")
    qk_pipe = [qk_compute(st0, interleave=True), qk_compute(qk_issue(1))]

    qt = NTD // 4
    for i in range(4):
        nc.sync.dma_start(out=wv_sb[:, i * qt:(i + 1) * qt, :],
                          in_=wvT_r[:, i * qt:(i + 1) * qt, :])

    # ---------------- v projection ----------------
    # hsT stationary, wvT moving -> token-major v_aug (bf16)
    for ts in range(NTS):
        ps = psU.tile([P, D], F32, tag="u", name="psv")
        for nch in range(NCH):
            for ti in range(NTD):
                nc.tensor.matmul(
                    ps[:, nch * CW:(nch + 1) * CW],
                    lhsT=hsT_sb[:, ti, ts * P:(ts + 1) * P],
                    rhs=wv_sb[:, ti, nch * CW:(nch + 1) * CW],
                    start=(ti == 0), stop=(ti == NTD - 1))
        if fast:
            nc.vector.tensor_copy(
                out=v_aug[:, ts, :, 0:HD],
                in_=ps.rearrange("p (h e) -> p h e", h=H))
        else:
            nc.vector.tensor_tensor(
                out=v_aug[:, ts, :, 0:HD],
                in0=ps.rearrange("p (h e) -> p h e", h=H),
                in1=bv_bc.rearrange("p (h e) -> p h e", h=H),
                op=ALU.add)

    # ---------------- attention (qk prefetched two head-pairs ahead) -------
    attnC = big16.tile([P, NTD, S], BF16, tag="big", name="attnC")
    hs_r = hs.rearrange("(t p) d -> p t d", p=P)
    ffn_w0 = None

    piece_q = []
    for hp in range(H // 2):
        qkh = qk_pipe.pop(0)
        if hp + 2 < H // 2:
            st_next = qk_issue(hp + 2)
            qk_pipe.append(st_next)
            piece_q.extend(qk_pieces(st_next))
        if hp == 0:
            # residual hs, issued now so it doesn't delay the wqk blocks
            # above it on the scalar queue (needed only from out-proj on)
            nc.sync.dma_start(out=hs_sb, in_=hs_r)
            if not fast:
                bo_bc = bcast_tile(wo_pool, bo, queue=nc.sync)
                for ts in range(NTS):
                    nc.vector.tensor_tensor(out=hs_sb[:, ts, :],
                                            in0=hs_sb[:, ts, :],
                                            in1=bo_bc, op=ALU.add)
        if hp == 1:
            nc.sync.dma_start(
                out=wo_sb, in_=woT.rearrange("(t p) d -> p t d", p=P))
            # FFN quarter-0 prefetch; the wv slot is recycled for w2 quarters
            ffn_w0 = load_quarter(0)
        for h in (2 * hp, 2 * hp + 1):
            r0 = (h % 2) * HD
            last_hp = (hp == H // 2 - 1)
            if last_hp:
                ps_at = [psQ.tile([P, CW], F32, tag="q5", name=f"atq{h}{c}")
                         for c in range(NCH)]
            else:
                at_full = psU.tile([P, S], F32, tag="u", name=f"at{h}")
                ps_at = [at_full[:, c * CW:(c + 1) * CW] for c in range(NCH)]
            def at_matmuls(tk, pr):
                for nch in range(NCH):
                    nc.tensor.matmul(
                        ps_at[nch][0:HD + 1, :],
                        lhsT=v_aug[:, tk, h, :],
                        rhs=pr[:, nch * CW:(nch + 1) * CW],
                        start=(tk == 0), stop=(tk == NTS - 1))

            # software-pipelined two deep: at(tk-2) runs behind sc(tk). The
            # exp->attn slack at one-deep (~126ns) almost exactly equals the
            # semaphore propagation cost (117ns), so every iteration paid
            # one sem-prop; two-deep makes the slack a full iteration.
            prs = []
            for tk in range(NTS):
                ps_sc = psU.tile([P, S], F32, tag="u", name=f"sc{h}")
                for nch in range(NCH):
                    nc.tensor.matmul(
                        ps_sc[:, nch * CW:(nch + 1) * CW],
                        lhsT=qkh["k"][r0:r0 + HD, tk * P:(tk + 1) * P],
                        rhs=qkh["q"][r0:r0 + HD, nch * CW:(nch + 1) * CW],
                        start=True, stop=True)
                if tk >= 3:
                    at_matmuls(tk - 3, prs[tk - 3])
                pr = probs_pool.tile([P, S], BF16, tag="pr", name=f"pr{h}")
                nc.scalar.activation(out=pr, in_=ps_sc, func=AFT.Exp)
                # paced at 14 of 16 tk slots per head-pair so the prefetch
                # work stretches through hp=6 (instead of running dry early
                # and leaving the last head-pairs with no PE slack work);
                # qk(hp+2) still completes before heads(hp+2) start. Two of
                # the seven pieces go between the pipeline-drain at() calls
                # below, whose exps otherwise expose a sem-prop each.
                if piece_q and tk <= 4:
                    piece_q.pop(0)()
                prs.append(pr)
            for tt in (NTS - 3, NTS - 2):
                at_matmuls(tt, prs[tt])
                if piece_q:
                    piece_q.pop(0)()
            at_matmuls(NTS - 1, prs[NTS - 1])
            if last_hp:
                # evict straight from PSUM per 512-chunk: shortest exposed
                # latency before the out-proj chain's final accumulations.
                for nch in range(NCH):
                    rrow = rr_pool.tile([1, CW], F32, tag="rr", name=f"rr{h}{nch}")
                    nc.vector.reciprocal(out=rrow, in_=ps_at[nch][HD:HD + 1, :])
                    bc = bc_pool.tile([P, CW], F32, tag="bc", name=f"bcr{h}{nch}")
                    nc.gpsimd.partition_broadcast(out_ap=bc, in_ap=rrow)
                    nc.vector.tensor_tensor(
                        out=attnC[r0:r0 + HD, hp, nch * CW:(nch + 1) * CW],
                        in0=ps_at[nch][0:HD, :], in1=bc[0:HD, :], op=ALU.mult)
            else:
                for nch in range(NCH):
                    stg = stg_pool.tile([P, CW], F32, tag="stg",
                                        name=f"stg{h}{nch}")
                    nc.vector.tensor_copy(out=stg[0:HD + 1, :],
                                          in_=ps_at[nch][0:HD + 1, :])
                    rrow = rr_pool.tile([1, CW], F32, tag="rr",
                                        name=f"rr{h}{nch}")
                    nc.vector.reciprocal(out=rrow, in_=stg[HD:HD + 1, :])
                    bc = bc_pool.tile([P, CW], F32, tag="bc", name=f"bcr{h}{nch}")
                    nc.gpsimd.partition_broadcast(out_ap=bc, in_ap=rrow)
                    nc.vector.tensor_tensor(
                        out=attnC[r0:r0 + HD, hp, nch * CW:(nch + 1) * CW],
                        in0=stg[0:HD, :], in1=bc[0:HD, :], op=ALU.mult)
    fused.close()
    vaug_pool.release()

    # ---------------- out proj + residual + LN1 + transpose ----------------
    # x1 is kept in bf16 (it feeds fc1/the residual through bf16 matmuls
    # anyway); the d-major x1T copy comes from the DMA XBAR transpose, off
    # the PE entirely (was 64 PE transposes + 64 psum-evict copies).
    x1b16 = x1bp.tile([P, NTS, D], BF16, tag="x1b", name="x1b16")
    # two separate half-tiles (s 0:512 / 512:1024) so fc1's first chains
    # depend only on the first four transposes, not ts=7's late one
    # (dependencies are tile-granular for the DMA-transpose writes)
    x1T_a = big16.tile([P, NTD, CW], BF16, tag="big", name="x1T_a")
    x1T_h = [x1T_a, x1bp.tile([P, NTD, CW], BF16, tag="x1tb", name="x1T_b")]

    with tc.tile_pool(name="lnc", bufs=1) as ln_pool, \
         tc.tile_pool(name="tmpC", bufs=3) as tmpC:
        if not fast:
            g1_bc = bcast_tile(ln_pool, g1, queue=nc.sync)
            b1_bc = bcast_tile(ln_pool, b1, queue=nc.sync)

        def transpose_issue(tt):
            # last two transposes ride the scalar queue: the sync-queue DMA
            # counting semaphore guarding x1T_a then never includes them, so
            # fc1's first chains are not held hostage to ts=7's transpose
            eng = nc.scalar if tt >= NTS - 2 else nc.sync
            eng.dma_start(
                out=x1T_h[tt // 4][:, :, (tt % 4) * P:(tt % 4 + 1) * P],
                in_=x1b16[:, tt, :], transpose=True)

        for ts in range(NTS):
            ps = psU.tile([P, D], F32, tag="u", name="pso")
            for nch in range(NCH):
                for td in range(NTD):
                    nc.tensor.matmul(
                        ps[:, nch * CW:(nch + 1) * CW],
                        lhsT=attnC[:, td, ts * P:(ts + 1) * P],
                        rhs=wo_sb[:, td, nch * CW:(nch + 1) * CW],
                        start=(td == 0), stop=(td == NTD - 1))
            # LN1 with per-512-chunk residual add + stats: the first chunk's
            # work starts while the second chunk's matmul chain still runs
            x0 = tmpC.tile([P, D], F32, tag="x0", name="x0")
            st1 = tmpC.tile([P, 2, 6], F32, tag="lnst1", name="lnst1")
            for nch in range(NCH):
                cs = slice(nch * CW, (nch + 1) * CW)
                nc.vector.tensor_tensor(out=x0[:, cs], in0=ps[:, cs],
                                        in1=hs_sb[:, ts, cs], op=ALU.add)
            for nch in range(NCH):
                cs = slice(nch * CW, (nch + 1) * CW)
                nc.vector.bn_stats(out=st1[:, nch, :], in_=x0[:, cs])
            mv = tmpC.tile([P, 2], F32, tag="lnmv", name="lnmv")
            nc.vector.bn_aggr(out=mv, in_=st1)
            nc.scalar.activation(out=mv[:, 1:2], in_=mv[:, 1:2], func=AFT.Sqrt,
                                 bias=eps_t, scale=1.0)
            nc.vector.reciprocal(out=mv[:, 1:2], in_=mv[:, 1:2])
            bneg = tmpC.tile([P, 1], F32, tag="lnbneg", name="lnbneg")
            nc.vector.tensor_scalar(out=bneg, in0=mv[:, 0:1],
                                    scalar1=mv[:, 1:2], scalar2=-1.0,
                                    op0=ALU.mult, op1=ALU.mult)
            for nch in range(NCH):
                cs = slice(nch * CW, (nch + 1) * CW)
                nc.scalar.activation(out=x1b16[:, ts, cs], in_=x0[:, cs],
                                     func=AFT.Identity, bias=bneg,
                                     scale=mv[:, 1:2])
                if not fast:
                    nc.gpsimd.tensor_tensor(out=x1b16[:, ts, cs],
                                            in0=x1b16[:, ts, cs],
                                            in1=g1_bc[:, cs], op=ALU.mult)
                    badd = nc.gpsimd if ts % 2 == 0 else nc.vector
                    badd.tensor_tensor(out=x1b16[:, ts, cs],
                                       in0=x1b16[:, ts, cs],
                                       in1=b1_bc[:, cs], op=ALU.add)
            # transpose issued two iterations late: its xhat has completed,
            # so the dma_start never parks on the queue head waiting
            if ts >= 2:
                transpose_issue(ts - 2)
        transpose_issue(NTS - 2)
        transpose_issue(NTS - 1)

    wo_pool.release()

    # ---------------- FFN (4 f-quarters, PSUM chains of 8) ----------------
    with tc.tile_pool(name="hTp", bufs=2) as hTp, \
         tc.tile_pool(name="fcb", bufs=1) as fcb_pool, \
         tc.tile_pool(name="tmpE", bufs=2) as tmpE:
        if not fast:
            f2b_bc = bcast_tile(fcb_pool, f2b, queue=nc.sync)
            g2_bc = bcast_tile(fcb_pool, g2, queue=nc.sync)
            b2_bc = bcast_tile(fcb_pool, b2, queue=nc.sync)
        out_r = out.rearrange("(t p) d -> p t d", p=P)
        out2 = None
        for q in range(NQ):
            w1, w2 = ffn_w0 if q == 0 else load_quarter(q)
            if q == 0:
                out2 = big32.tile([P, NTS, D], F32, tag="big", name="out2")
            hT_q = hTp.tile([P, FQT, S], BF16, tag="hT", name="hT_q")

            def fc1_chain(ft, nch, ps):
                for td in range(NTD):
                    nc.tensor.matmul(
                        ps[:, nch * CW:(nch + 1) * CW],
                        lhsT=w1[:, td, ft * P:(ft + 1) * P],
                        rhs=x1T_h[nch][:, td, :],
                        start=(td == 0), stop=(td == NTD - 1))

            def fc1_gelu(ft, ps):
                tf = q * FQT + ft
                if fast:
                    nc.scalar.activation(out=hT_q[:, ft, :], in_=ps,
                                         func=AFT.Gelu)
                else:
                    nc.scalar.activation(out=hT_q[:, ft, :], in_=ps,
                                         func=AFT.Gelu,
                                         bias=f1b_t[:, tf:tf + 1], scale=1.0)

            if q == 0:
                # nch1 chains trail three ft slots behind nch0: the second
                # x1T half (ts=7's transpose, finished only ~5us into the
                # FFN) is then never waited on by the PE.
                lag = 3
                ftps = {}
                for ft in range(FQT + lag):
                    if ft < FQT:
                        ftps[ft] = psU.tile([P, S], F32, tag="u", name="psh")
                        fc1_chain(ft, 0, ftps[ft])
                    if ft >= lag:
                        fc1_chain(ft - lag, 1, ftps[ft - lag])
                        fc1_gelu(ft - lag, ftps[ft - lag])
            else:
                for ft in range(FQT):
                    ps = psU.tile([P, S], F32, tag="u", name="psh")
                    for nch in range(NCH):
                        fc1_chain(ft, nch, ps)
                    fc1_gelu(ft, ps)
            last_q = q == NQ - 1

            def q3_preload(ts):
                # preload out2 + x1 residual into PSUM; the final chain then
                # accumulates on top (start=False) and LN2 stats / xhat read
                # the finished PSUM directly — no SBUF evict-add is left on
                # the critical tail.
                pst = psU.tile([P, D], F32, tag="u", name="pso2")
                nc.vector.tensor_tensor(out=pst, in0=out2[:, ts, :],
                                        in1=x1b16[:, ts, :], op=ALU.add)
                return pst

            if last_q:
                ps_next = q3_preload(0)
            for ts in range(NTS):
                ps = ps_next if last_q else psU.tile([P, D], F32, tag="u",
                                                     name="pso2")
                for nch in range(NCH):
                    for ft in range(FQT):
                        nc.tensor.matmul(
                            ps[:, nch * CW:(nch + 1) * CW],
                            lhsT=hT_q[:, ft, ts * P:(ts + 1) * P],
                            rhs=w2[:, ft, nch * CW:(nch + 1) * CW],
                            start=(ft == 0 and not last_q),
                            stop=(ft == FQT - 1),
                            skip_group_check=last_q)
                if last_q and ts + 1 < NTS:
                    # next tile's preload issued BEFORE this tile's LN2: the
                    # 1127ns DVE preload otherwise queues behind the full LN2
                    # chain, delaying the final tile's stats at the tail
                    ps_next = q3_preload(ts + 1)
                if last_q:
                    # LN2 in quarter-chunks: the trailing chunk's stats/xhat/
                    # store are each ~350ns, minimizing the serial tail after
                    # the very last matmul.
                    NCK = 2
                    CKW = D // NCK
                    st2 = tmpE.tile([P, NCK, 6], F32, tag="lnst2", name="lnst2")
                    for ck in range(NCK):
                        cs = slice(ck * CKW, (ck + 1) * CKW)
                        nc.vector.bn_stats(out=st2[:, ck, :], in_=ps[:, cs])
                    mv = tmpE.tile([P, 2], F32, tag="lnmv2", name="lnmv2")
                    nc.vector.bn_aggr(out=mv, in_=st2)
                    nc.scalar.activation(out=mv[:, 1:2], in_=mv[:, 1:2],
                                         func=AFT.Sqrt, bias=eps_t, scale=1.0)
                    nc.vector.reciprocal(out=mv[:, 1:2], in_=mv[:, 1:2])
                    bneg = tmpE.tile([P, 1], F32, tag="lnbn2", name="lnbn2")
                    nc.vector.tensor_scalar(out=bneg, in0=mv[:, 0:1],
                                            scalar1=mv[:, 1:2], scalar2=-1.0,
                                            op0=ALU.mult, op1=ALU.mult)
                    yt = tmpE.tile([P, D], F32, tag="ye", name="ye")
                    for ck in range(NCK):
                        cs = slice(ck * CKW, (ck + 1) * CKW)
                        if fast and ts == NTS - 1 and ck == NCK - 1:
                            # last tile's second xhat half on DVE, in
                            # parallel with ACT's first half: xhat =
                            # ps*rstd + (-mean*rstd)
                            nc.vector.tensor_scalar(
                                out=yt[:, cs], in0=ps[:, cs],
                                scalar1=mv[:, 1:2], scalar2=bneg,
                                op0=ALU.mult, op1=ALU.add)
                        else:
                            nc.scalar.activation(out=yt[:, cs], in_=ps[:, cs],
                                                 func=AFT.Identity, bias=bneg,
                                                 scale=mv[:, 1:2])
                        if not fast:
                            nc.gpsimd.tensor_tensor(out=yt[:, cs], in0=yt[:, cs],
                                                    in1=g2_bc[:, cs], op=ALU.mult)
                            badd = nc.gpsimd if ts % 2 == 0 else nc.vector
                            badd.tensor_tensor(out=yt[:, cs], in0=yt[:, cs],
                                               in1=b2_bc[:, cs], op=ALU.add)
                        nc.sync.dma_start(out=out_r[:, ts, cs], in_=yt[:, cs])
                elif q == 0:
                    if fast:
                        nc.vector.tensor_copy(out=out2[:, ts, :], in_=ps)
                    else:
                        nc.vector.tensor_tensor(out=out2[:, ts, :], in0=ps,
                                                in1=f2b_bc, op=ALU.add)
                else:
                    nc.vector.tensor_tensor(out=out2[:, ts, :], in0=ps,
                                            in1=out2[:, ts, :], op=ALU.add)

    psQ.release()
    psU.release()
    x1bp.release()
    big32.release()
    big16.release()
    f1wp.release()
    f2wp.release()
    const.release()


# ---------------------------------------------------------------------------
# Full-input entry point: data-parallel over batch across 8 NeuronCores.
# ---------------------------------------------------------------------------
import numpy as np
import ml_dtypes
from concourse import bass_utils

B = 8
SCALING = HD ** -0.5
BF = ml_dtypes.bfloat16

_NC_CACHE = {}


def _get_nc(fast=True):
    if fast not in _NC_CACHE:
        _NC_CACHE[fast] = build(fast=fast)
    return _NC_CACHE[fast]


def _blk(wt):
    """[D, D] -> (H/2, P, D) with wblk[hp][p, ti*P+c] = wt[ti*P+p, hp*P+c]."""
    a = np.asarray(wt, dtype=np.float32).reshape(D // P, P, H // 2, P)
    return np.ascontiguousarray(
        a.transpose(2, 1, 0, 3).reshape(H // 2, P, D)).astype(BF)


def _prep_core_inputs(b_hs, w):
    c = np.ascontiguousarray
    f = np.float32

    def a(x):
        return c(np.asarray(x)).astype(f, copy=False)

    def ab(x):
        return c(np.asarray(x, dtype=f)).astype(BF)

    return {
        "hsT": ab(np.asarray(b_hs).T),
        "hs": a(b_hs),
        "wqb": _blk(np.asarray(w["wq"], dtype=f).T * SCALING),
        "wkb": _blk(np.asarray(w["wk"], dtype=f).T),
        "wvT": ab(np.asarray(w["wv"], dtype=f).T),
        "woT": ab(np.asarray(w["wo"], dtype=f).T),
        "bq": a(np.asarray(w["bq"], dtype=f) * SCALING),
        "bk": a(w["bk"]),
        "bv": a(w["bv"]),
        "bo": a(w["bo"]),
        "g1": a(w["ln1_g"]),
        "b1": a(w["ln1_b"]),
        "g2": a(w["ln2_g"]),
        "b2": a(w["ln2_b"]),
        "f1w": ab(np.asarray(w["fc1_w"], dtype=f).T),
        "f1b": a(w["fc1_b"]),
        "f2w": ab(np.asarray(w["fc2_w"], dtype=f).T),
        "f2b": a(w["fc2_b"]),
    }


def kernel(**inputs):
    """Takes full unsharded inputs (setup_inputs() keys), returns [B, S, D]."""
    w = {k: np.asarray(v) for k, v in inputs.items()}
    hs_all = w["hidden_states"]
    assert hs_all.shape == (B, S, D), hs_all.shape
    fast = all(
        np.all(np.asarray(w[k]) == 0.0)
        for k in ("bq", "bk", "bv", "bo", "fc1_b", "fc2_b", "ln1_b", "ln2_b")
    ) and all(np.all(np.asarray(w[k]) == 1.0) for k in ("ln1_g", "ln2_g"))
    nc = _get_nc(fast)
    in_maps = [_prep_core_inputs(hs_all[c], w) for c in range(B)]
    res = bass_utils.run_bass_kernel_spmd(nc, in_maps, core_ids=list(range(B)))
    out_full = np.stack([res.results[c]["out"] for c in range(B)])
    return out_full.astype(np.float32, copy=False)
